# revision 35
# baseline (speedup 1.0000x reference)
"""Causal self-attention (B=4, T=2048, E=1024, H=16) on 8 trn2 NeuronCores.

Sharding: core c -> (batch b = c // 2, head-group hg = c % 2); each core owns
one batch element and 8 of the 16 heads (data parallel on B, tensor parallel
on heads).  No cross-core communication.

Per-core device program (SPMD, same NEFF on all 8 cores), interleaved per
512-token block tb: QKV projection for tb, then attention for query block
I = tb (causal -> only needs k/v from blocks <= tb):
  qT,kT [c,t]-layout (2 heads packed per 128-partition tile), bias on DVE
  v     [t,c]-layout with a ones column per head, bias via K=1 matmul
  attention (all matmuls in 64-row PE tiling mode, no mode switches):
    St[j,i] strip [A|B]: QK row-tile pair computes 2 heads concurrently
    Pt = exp(St/8) on ScalarE (one instr per head pair), causal mask via a
         width-trimmed gpsimd affine_select on diagonal tiles only
    Yt[d|sum, i]: PV row-tile pair (j split 64+64) -> 2 psum partials,
         summed on DVE; softmax denominators come out as row 64
    y = Yt[:64] * (1/Yt[64]); the broadcast of the reciprocal across
        partitions is a K=1 matmul (ones^T @ recip)
Output written as yT [c, t]; the host transposes and concatenates.
"""

import sys

sys.path.insert(0, "/opt/trn_rl_repo")

import numpy as np

N_CORES = 8
B, T, E = 4, 2048, 1024
H, D = 16, 64
C = E                 # q/k/v channel count (4th qkv chunk unused)
HPC = H // 2          # heads per core
CC = HPC * D          # per-core channels = 512
ES = E // 128         # 8 e-tiles (contraction)
TB = T // 512         # 4 t/i blocks of 512
NJ = T // 128         # 16 j-tiles of 128
PAIRS = HPC // 2      # 4 head pairs per core

_cache = {}


def _build_nc():
    import concourse.mybir as mybir
    import concourse.tile as tile
    from concourse import bacc

    f32 = mybir.dt.float32
    f32r = mybir.dt.float32r
    Act = mybir.ActivationFunctionType
    is_ge = mybir.AluOpType.is_ge

    nc = bacc.Bacc("TRN2", target_bir_lowering=False, debug=False)

    xT = nc.dram_tensor("xT", [E, T], f32r, kind="ExternalInput").ap()
    w_qk = nc.dram_tensor("w_qk", [E, 2 * CC], f32r, kind="ExternalInput").ap()
    w_v = nc.dram_tensor("w_v", [E, CC], f32r, kind="ExternalInput").ap()
    b_qk = nc.dram_tensor("b_qk", [128, 8], f32, kind="ExternalInput").ap()
    b_v = nc.dram_tensor("b_v", [1, CC], f32r, kind="ExternalInput").ap()
    ones_d = nc.dram_tensor("ones_d", [1, 128], f32r, kind="ExternalInput").ap()
    yT = nc.dram_tensor("yT", [CC, T], f32, kind="ExternalOutput").ap()

    with tile.TileContext(nc) as tc:
        with (
            tc.tile_pool(name="persist", bufs=1) as pp,
            tc.tile_pool(name="psum", bufs=1, space="PSUM") as psp,
            tc.tile_pool(name="xpool", bufs=2) as xp,
            tc.tile_pool(name="ptpool", bufs=3) as ptp,
            tc.tile_pool(name="opool", bufs=1) as op,
            tc.tile_pool(name="dpool", bufs=2, space="DRAM") as dp,
        ):
            # ---- persistent SBUF state ----
            qk_sb = [pp.tile([128, T], f32r, name=f"qk{ct}") for ct in range(8)]
            # v plus a ones column per head: [t-part, head, t-tile, 65]
            v1_sb = pp.tile([128, HPC, NJ, D + 1], f32r, name="v1")
            bqk_sb = pp.tile([128, 8], f32, name="bqk")
            bv_sb = pp.tile([1, CC], f32r, name="bv")
            ones_sb = pp.tile([1, 128], f32r, name="ones")
            wqk_t = []
            wv_t = []

            # input DMAs: x(tb0) first so the first matmul group can start,
            # then weights, then the small vectors
            xs_tb = {}

            def load_x(tb):
                tsl = slice(tb * 512, (tb + 1) * 512)
                xs = []
                for e in range(ES):
                    xe = xp.tile([128, 512], f32r, tag=f"x{e}", name=f"x{e}_{tb}")
                    nc.sync.dma_start(out=xe, in_=xT[e * 128 : (e + 1) * 128, tsl])
                    xs.append(xe)
                xs_tb[tb] = xs

            # small constants first, then x/w interleaved per e-tile so the
            # first matmul accumulation group can finish as early as possible
            nc.sync.dma_start(out=bqk_sb, in_=b_qk)
            nc.sync.dma_start(out=bv_sb, in_=b_v)
            nc.sync.dma_start(out=ones_sb, in_=ones_d)
            tsl0 = slice(0, 512)
            xs0 = []
            for e in range(ES):
                xe = xp.tile([128, 512], f32r, tag=f"x{e}", name=f"x{e}_0")
                nc.sync.dma_start(out=xe, in_=xT[e * 128 : (e + 1) * 128, tsl0])
                xs0.append(xe)
                wqk = pp.tile([128, 2 * CC], f32r, name=f"wqk{e}")
                nc.sync.dma_start(out=wqk, in_=w_qk[e * 128 : (e + 1) * 128, :])
                wqk_t.append(wqk)
            xs_tb[0] = xs0
            for e in range(ES):
                wv = pp.tile([128, CC], f32r, name=f"wv{e}")
                nc.sync.dma_start(out=wv, in_=w_v[e * 128 : (e + 1) * 128, :])
                wv_t.append(wv)
            ones_bc = _bcast_ap(ones_d, 128)
            nc.sync.dma_start(out=v1_sb[:, :, :, D : D + 1], in_=ones_bc)

            def qkv_group_qk(tb, ct):
                tsl = slice(tb * 512, (tb + 1) * 512)
                xs = xs_tb[tb]
                ps = psp.tile([128, 512], f32, tag="st", bufs=2,
                              name=f"psqk{ct}_{tb}")
                for e in range(ES):
                    nc.tensor.matmul(
                        ps,
                        wqk_t[e][:, ct * 128 : (ct + 1) * 128],
                        xs[e],
                        start=(e == 0),
                        stop=(e == ES - 1),
                    )
                nc.scalar.activation(
                    qk_sb[ct][:, tsl], ps, Act.Identity,
                    bias=bqk_sb[:, ct : ct + 1], scale=1.0)

            def qkv_group_v(tb, k4):
                xs = xs_tb[tb]
                tt = tb * 4 + k4
                psv = psp.tile([128, 512], f32, tag="st", bufs=2,
                               name=f"psv{tt}")
                nc.tensor.matmul(
                    psv, ones_sb, bv_sb,
                    start=True, stop=False, skip_group_check=True,
                )
                for e in range(ES):
                    nc.tensor.matmul(
                        psv,
                        xs[e][:, k4 * 128 : (k4 + 1) * 128],
                        wv_t[e],
                        start=False,
                        stop=(e == ES - 1),
                        skip_group_check=True,
                    )
                nc.vector.tensor_copy(
                    v1_sb[:, :, tt, 0:D],
                    psv.rearrange("p (h d) -> p h d", d=D),
                )

            def attn_block(I):
                isl = slice(I * 512, (I + 1) * 512)
                nj = 4 * I + 4  # causal j-tiles for this i-block
                yts = {}
                pts = {}

                def alloc_yt(pr):
                    yts[pr] = [
                        psp.tile([D + 1, 512], f32, tag=f"yt{n}",
                                 name=f"yt{n}_{pr}_{I}")
                        for n in ("A0", "A1", "B0", "B1")
                    ]

                def qk_exp(pr, J):
                    qt = qk_sb[pr]
                    kt = qk_sb[4 + pr]
                    jsl = slice(J * 128, (J + 1) * 128)
                    st = psp.tile([128, 1024], f32, tag="st", bufs=2,
                                  name=f"st{pr}_{I}_{J}")
                    # QK row-tile pair: head A rows 0-63, head B 64-127
                    nc.tensor.matmul(
                        st[:, 0:512], kt[0:64, jsl], qt[0:64, isl],
                        tile_position=(0, 0),
                    )
                    nc.tensor.matmul(
                        st[:, 512:1024], kt[64:128, jsl], qt[64:128, isl],
                        tile_position=(64, 0),
                    )
                    pt = ptp.tile([128, 1024], f32r, tag="pt",
                                  name=f"pt{pr}_{I}_{J}")
                    nc.scalar.activation(pt, st, Act.Exp, scale=0.125)
                    r = J - 4 * I
                    if r >= 0:  # diagonal tile: causal mask, trimmed width
                        w = (r + 1) * 128
                        for off in (0, 512):
                            # keep where (512I + y) - (128J + x) >= 0
                            nc.gpsimd.affine_select(
                                out=pt[:, off : off + w],
                                in_=pt[:, off : off + w],
                                compare_op=is_ge,
                                fill=0.0,
                                base=-128 * r,
                                pattern=[[1, w]],
                                channel_multiplier=-1,
                            )
                    pts[(pr, J)] = pt

                def pv(pr, J):
                    pt = pts.pop((pr, J))
                    ytA0, ytA1, ytB0, ytB1 = yts[pr]
                    first, last = (J == 0), (J == nj - 1)
                    # PV row-tile pairs (j contraction split 64+64)
                    nc.tensor.matmul(
                        ytA0, v1_sb[0:64, 2 * pr, J, :], pt[0:64, 0:512],
                        tile_position=(0, 0),
                        start=first, stop=last, skip_group_check=True,
                    )
                    nc.tensor.matmul(
                        ytA1, v1_sb[64:128, 2 * pr, J, :], pt[64:128, 0:512],
                        tile_position=(64, 0),
                        start=first, stop=last, skip_group_check=True,
                    )
                    nc.tensor.matmul(
                        ytB0, v1_sb[0:64, 2 * pr + 1, J, :], pt[0:64, 512:1024],
                        tile_position=(0, 0),
                        start=first, stop=last, skip_group_check=True,
                    )
                    nc.tensor.matmul(
                        ytB1, v1_sb[64:128, 2 * pr + 1, J, :],
                        pt[64:128, 512:1024],
                        tile_position=(64, 0),
                        start=first, stop=last, skip_group_check=True,
                    )

                def out_stage(pr):
                    ytA0, ytA1, ytB0, ytB1 = yts.pop(pr)
                    # ---- normalize + emit [128 rows = 2 heads, 512] ----
                    ystage = op.tile([128, 512], f32, tag="ystage", bufs=2,
                                     name=f"ys{pr}_{I}")
                    rec2 = op.tile([33, 512], f32, tag="rec2",
                                   name=f"rec2{pr}_{I}")
                    sum2 = op.tile([33, 512], f32, tag="sum2",
                                   name=f"sum2{pr}_{I}")
                    sA = op.tile([D + 1, 512], f32, tag="sA", name=f"sA{pr}_{I}")
                    sB = op.tile([D + 1, 512], f32, tag="sB", name=f"sB{pr}_{I}")
                    recA, recB = rec2[0:1, :], rec2[32:33, :]
                    sumA, sumB = sum2[0:1, :], sum2[32:33, :]
                    nc.vector.tensor_copy(sA, ytA1)
                    nc.vector.tensor_copy(sB, ytB1)
                    nc.vector.tensor_add(ystage[0:64, :], ytA0[0:D, :], sA[0:D, :])
                    nc.vector.tensor_add(ystage[64:128, :], ytB0[0:D, :],
                                         sB[0:D, :])
                    nc.vector.tensor_add(sumA, ytA0[D : D + 1, :],
                                         sA[D : D + 1, :])
                    nc.vector.tensor_add(sumB, ytB0[D : D + 1, :],
                                         sB[D : D + 1, :])
                    nc.vector.reciprocal(recA, sumA)
                    nc.vector.reciprocal(recB, sumB)
                    # broadcast 1/sum across partitions via a DRAM bounce
                    # (keeps the PE stream free of output-stage work)
                    recA_d = dp.tile([1, 512], f32, tag="recA_d",
                                     name=f"recAd{pr}_{I}")
                    recB_d = dp.tile([1, 512], f32, tag="recB_d",
                                     name=f"recBd{pr}_{I}")
                    rbc2 = op.tile([128, 512], f32, tag="rbc2",
                                   name=f"rbc2{pr}_{I}")
                    rbcA, rbcB = rbc2[0:64, :], rbc2[64:128, :]
                    nc.sync.dma_start(out=recA_d, in_=recA)
                    nc.sync.dma_start(out=recB_d, in_=recB)
                    nc.sync.dma_start(out=rbcA, in_=_bcast_ap(recA_d, 64))
                    nc.sync.dma_start(out=rbcB, in_=_bcast_ap(recB_d, 64))
                    nc.vector.tensor_mul(ystage[0:64, :], ystage[0:64, :], rbcA)
                    nc.vector.tensor_mul(ystage[64:128, :], ystage[64:128, :],
                                         rbcB)
                    nc.sync.dma_start(
                        out=yT[pr * 128 : (pr + 1) * 128, isl], in_=ystage)

                # 1-stage software pipeline across the whole block: QK(k+1)
                # issues before PV(k) so the PE never sits behind a PV that
                # is waiting on exp
                items = [(pr, J) for pr in range(PAIRS) for J in range(nj)]
                prev = None
                for it in items:
                    if it[1] == 0:
                        alloc_yt(it[0])
                    qk_exp(*it)
                    if prev is not None:
                        pv(*prev)
                        if prev[1] == nj - 1:
                            out_stage(prev[0])
                    prev = it
                pv(*prev)
                out_stage(prev[0])

            # schedule: per t-block, QKV projection then attention I = tb
            # (causal: block I only needs k/v from t-blocks <= I)
            for g in range(12):
                (qkv_group_qk(0, g) if g < 8 else qkv_group_v(0, g - 8))
            for I in range(TB):
                nxt = []
                if I + 1 < TB:
                    load_x(I + 1)
                    nxt = [(qkv_group_qk, I + 1, g) for g in range(8)] + [
                        (qkv_group_v, I + 1, g) for g in range(4)
                    ]
                attn_block(I)
                for fn, a, b in nxt:
                    fn(a, b)
    nc.compile()
    return nc


def _bcast_ap(src_ap, nparts):
    """Partition-broadcast view of a [1, N] DRAM AP -> [nparts, N]."""
    import concourse.bass as bass

    return bass.AP(
        tensor=src_ap.tensor,
        offset=src_ap.offset,
        ap=[[0, nparts]] + list(src_ap.ap)[1:],
    )


def get_nc():
    if "nc" not in _cache:
        _cache["nc"] = _build_nc()
    return _cache["nc"]


def shard_inputs(x, w_attn, b_attn):
    """Full inputs -> per-core input maps (host-side slicing/transposition)."""
    x = np.asarray(x, dtype=np.float32)
    w = np.asarray(w_attn, dtype=np.float32)
    bb = np.asarray(b_attn, dtype=np.float32)
    in_maps = []
    for core in range(N_CORES):
        b, hg = core // 2, core % 2
        r0 = hg * CC  # first q row for this head group
        w_qk = np.ascontiguousarray(
            np.concatenate([w[r0 : r0 + CC, :], w[C + r0 : C + r0 + CC, :]], axis=0).T
        )
        w_v = np.ascontiguousarray(w[2 * C + r0 : 2 * C + r0 + CC, :].T)
        b_qk = np.stack(
            [bb[r0 + ct * 128 : r0 + (ct + 1) * 128] for ct in range(4)]
            + [bb[C + r0 + ct * 128 : C + r0 + (ct + 1) * 128] for ct in range(4)],
            axis=1,
        ).astype(np.float32)
        b_v = bb[2 * C + r0 : 2 * C + r0 + CC].reshape(1, CC).astype(np.float32)
        in_maps.append(
            {
                "xT": np.ascontiguousarray(x[b].T),
                "w_qk": w_qk,
                "w_v": w_v,
                "b_qk": np.ascontiguousarray(b_qk),
                "b_v": np.ascontiguousarray(b_v),
                "ones_d": np.ones((1, 128), dtype=np.float32),
            }
        )
    return in_maps


def run(in_maps, trace=False, **kw):
    from concourse import bass_utils

    nc = get_nc()
    return bass_utils.run_bass_kernel_spmd(
        nc, in_maps, core_ids=list(range(N_CORES)), trace=trace, **kw
    )


def gather_output(results):
    y = np.empty((B, T, E), dtype=np.float32)
    for core in range(N_CORES):
        b, hg = core // 2, core % 2
        y[b, :, hg * CC : (hg + 1) * CC] = results[core]["yT"].T
    return y


def kernel(x, w_attn, b_attn):
    in_maps = shard_inputs(x, w_attn, b_attn)
    res = run(in_maps, trace=False)
    return gather_output(res.results)


# revision 36
# speedup vs baseline: 1.0381x; 1.0381x over previous
"""Causal self-attention (B=4, T=2048, E=1024, H=16) on 8 trn2 NeuronCores.

Sharding: core c -> (batch b = c // 2, head-group hg = c % 2); each core owns
one batch element and 8 of the 16 heads (data parallel on B, tensor parallel
on heads).  No cross-core communication.

Per-core device program (SPMD, same NEFF on all 8 cores), interleaved per
512-token block tb: QKV projection for tb, then attention for query block
I = tb (causal -> only needs k/v from blocks <= tb):
  qT,kT [c,t]-layout (2 heads packed per 128-partition tile), bias on DVE
  v     [t,c]-layout with a ones column per head, bias via K=1 matmul
  attention (all matmuls in 64-row PE tiling mode, no mode switches):
    St[j,i] strip [A|B]: QK row-tile pair computes 2 heads concurrently
    Pt = exp(St/8) on ScalarE (one instr per head pair), causal mask via a
         width-trimmed gpsimd affine_select on diagonal tiles only
    Yt[d|sum, i]: PV row-tile pair (j split 64+64) -> 2 psum partials,
         summed on DVE; softmax denominators come out as row 64
    y = Yt[:64] * (1/Yt[64]); the broadcast of the reciprocal across
        partitions is a K=1 matmul (ones^T @ recip)
Output written as yT [c, t]; the host transposes and concatenates.
"""

import sys

sys.path.insert(0, "/opt/trn_rl_repo")

import numpy as np

N_CORES = 8
B, T, E = 4, 2048, 1024
H, D = 16, 64
C = E                 # q/k/v channel count (4th qkv chunk unused)
HPC = H // 2          # heads per core
CC = HPC * D          # per-core channels = 512
ES = E // 128         # 8 e-tiles (contraction)
TB = T // 512         # 4 t/i blocks of 512
NJ = T // 128         # 16 j-tiles of 128
PAIRS = HPC // 2      # 4 head pairs per core

_cache = {}


def _build_nc():
    import concourse.mybir as mybir
    import concourse.tile as tile
    from concourse import bacc

    f32 = mybir.dt.float32
    f32r = mybir.dt.float32r
    Act = mybir.ActivationFunctionType
    is_ge = mybir.AluOpType.is_ge

    nc = bacc.Bacc("TRN2", target_bir_lowering=False, debug=False)

    xT = nc.dram_tensor("xT", [E, T], f32r, kind="ExternalInput").ap()
    w_qk = nc.dram_tensor("w_qk", [E, 2 * CC], f32r, kind="ExternalInput").ap()
    w_v = nc.dram_tensor("w_v", [E, CC], f32r, kind="ExternalInput").ap()
    b_qk = nc.dram_tensor("b_qk", [128, 8], f32, kind="ExternalInput").ap()
    b_v = nc.dram_tensor("b_v", [1, CC], f32r, kind="ExternalInput").ap()
    ones_d = nc.dram_tensor("ones_d", [1, 128], f32r, kind="ExternalInput").ap()
    yT = nc.dram_tensor("yT", [CC, T], f32, kind="ExternalOutput").ap()

    with tile.TileContext(nc) as tc:
        with (
            tc.tile_pool(name="persist", bufs=1) as pp,
            tc.tile_pool(name="psum", bufs=1, space="PSUM") as psp,
            tc.tile_pool(name="xpool", bufs=2) as xp,
            tc.tile_pool(name="ptpool", bufs=4) as ptp,
            tc.tile_pool(name="opool", bufs=1) as op,
            tc.tile_pool(name="dpool", bufs=2, space="DRAM") as dp,
        ):
            # ---- persistent SBUF state ----
            qk_sb = [pp.tile([128, T], f32r, name=f"qk{ct}") for ct in range(8)]
            # v plus a ones column per head: [t-part, head, t-tile, 65]
            v1_sb = pp.tile([128, HPC, NJ, D + 1], f32r, name="v1")
            bqk_sb = pp.tile([128, 8], f32, name="bqk")
            bv_sb = pp.tile([1, CC], f32r, name="bv")
            ones_sb = pp.tile([1, 128], f32r, name="ones")
            wqk_t = []
            wv_t = []

            # input DMAs: x(tb0) first so the first matmul group can start,
            # then weights, then the small vectors
            xs_tb = {}

            def load_x(tb):
                tsl = slice(tb * 512, (tb + 1) * 512)
                xs = []
                for e in range(ES):
                    xe = xp.tile([128, 512], f32r, tag=f"x{e}",
                                 bufs=(2 if e < 4 else 1), name=f"x{e}_{tb}")
                    nc.sync.dma_start(out=xe, in_=xT[e * 128 : (e + 1) * 128, tsl])
                    xs.append(xe)
                xs_tb[tb] = xs

            # small constants first, then x/w interleaved per e-tile so the
            # first matmul accumulation group can finish as early as possible
            nc.sync.dma_start(out=bqk_sb, in_=b_qk)
            nc.sync.dma_start(out=bv_sb, in_=b_v)
            nc.sync.dma_start(out=ones_sb, in_=ones_d)
            tsl0 = slice(0, 512)
            xs0 = []
            for e in range(ES):
                xe = xp.tile([128, 512], f32r, tag=f"x{e}",
                             bufs=(2 if e < 4 else 1), name=f"x{e}_0")
                nc.sync.dma_start(out=xe, in_=xT[e * 128 : (e + 1) * 128, tsl0])
                xs0.append(xe)
                wqk = pp.tile([128, 2 * CC], f32r, name=f"wqk{e}")
                nc.sync.dma_start(out=wqk, in_=w_qk[e * 128 : (e + 1) * 128, :])
                wqk_t.append(wqk)
            xs_tb[0] = xs0
            for e in range(ES):
                wv = pp.tile([128, CC], f32r, name=f"wv{e}")
                nc.sync.dma_start(out=wv, in_=w_v[e * 128 : (e + 1) * 128, :])
                wv_t.append(wv)
            ones_bc = _bcast_ap(ones_d, 128)
            nc.sync.dma_start(out=v1_sb[:, :, :, D : D + 1], in_=ones_bc)

            def qkv_group_qk(tb, ct):
                tsl = slice(tb * 512, (tb + 1) * 512)
                xs = xs_tb[tb]
                ps = psp.tile([128, 512], f32, tag="st", bufs=2,
                              name=f"psqk{ct}_{tb}")
                for e in range(ES):
                    nc.tensor.matmul(
                        ps,
                        wqk_t[e][:, ct * 128 : (ct + 1) * 128],
                        xs[e],
                        start=(e == 0),
                        stop=(e == ES - 1),
                    )
                nc.scalar.activation(
                    qk_sb[ct][:, tsl], ps, Act.Identity,
                    bias=bqk_sb[:, ct : ct + 1], scale=1.0)

            def qkv_group_v(tb, k4):
                xs = xs_tb[tb]
                tt = tb * 4 + k4
                psv = psp.tile([128, 512], f32, tag="st", bufs=2,
                               name=f"psv{tt}")
                nc.tensor.matmul(
                    psv, ones_sb, bv_sb,
                    start=True, stop=False, skip_group_check=True,
                )
                for e in range(ES):
                    nc.tensor.matmul(
                        psv,
                        xs[e][:, k4 * 128 : (k4 + 1) * 128],
                        wv_t[e],
                        start=False,
                        stop=(e == ES - 1),
                        skip_group_check=True,
                    )
                nc.vector.tensor_copy(
                    v1_sb[:, :, tt, 0:D],
                    psv.rearrange("p (h d) -> p h d", d=D),
                )

            def attn_block(I):
                isl = slice(I * 512, (I + 1) * 512)
                nj = 4 * I + 4  # causal j-tiles for this i-block
                yts = {}
                pts = {}

                def alloc_yt(pr):
                    yts[pr] = [
                        psp.tile([D + 1, 512], f32, tag=f"yt{n}",
                                 name=f"yt{n}_{pr}_{I}")
                        for n in ("A0", "A1", "B0", "B1")
                    ]

                def qk_exp(pr, J):
                    qt = qk_sb[pr]
                    kt = qk_sb[4 + pr]
                    jsl = slice(J * 128, (J + 1) * 128)
                    st = psp.tile([128, 1024], f32, tag="st", bufs=2,
                                  name=f"st{pr}_{I}_{J}")
                    # QK row-tile pair: head A rows 0-63, head B 64-127
                    nc.tensor.matmul(
                        st[:, 0:512], kt[0:64, jsl], qt[0:64, isl],
                        tile_position=(0, 0),
                    )
                    nc.tensor.matmul(
                        st[:, 512:1024], kt[64:128, jsl], qt[64:128, isl],
                        tile_position=(64, 0),
                    )
                    pt = ptp.tile([128, 1024], f32r, tag="pt",
                                  name=f"pt{pr}_{I}_{J}")
                    nc.scalar.activation(pt, st, Act.Exp, scale=0.125)
                    r = J - 4 * I
                    if r >= 0:  # diagonal tile: causal mask, trimmed width
                        w = (r + 1) * 128
                        for off in (0, 512):
                            # keep where (512I + y) - (128J + x) >= 0
                            nc.gpsimd.affine_select(
                                out=pt[:, off : off + w],
                                in_=pt[:, off : off + w],
                                compare_op=is_ge,
                                fill=0.0,
                                base=-128 * r,
                                pattern=[[1, w]],
                                channel_multiplier=-1,
                            )
                    pts[(pr, J)] = pt

                def pv(pr, J):
                    pt = pts.pop((pr, J))
                    ytA0, ytA1, ytB0, ytB1 = yts[pr]
                    first, last = (J == 0), (J == nj - 1)
                    # PV row-tile pairs (j contraction split 64+64)
                    nc.tensor.matmul(
                        ytA0, v1_sb[0:64, 2 * pr, J, :], pt[0:64, 0:512],
                        tile_position=(0, 0),
                        start=first, stop=last, skip_group_check=True,
                    )
                    nc.tensor.matmul(
                        ytA1, v1_sb[64:128, 2 * pr, J, :], pt[64:128, 0:512],
                        tile_position=(64, 0),
                        start=first, stop=last, skip_group_check=True,
                    )
                    nc.tensor.matmul(
                        ytB0, v1_sb[0:64, 2 * pr + 1, J, :], pt[0:64, 512:1024],
                        tile_position=(0, 0),
                        start=first, stop=last, skip_group_check=True,
                    )
                    nc.tensor.matmul(
                        ytB1, v1_sb[64:128, 2 * pr + 1, J, :],
                        pt[64:128, 512:1024],
                        tile_position=(64, 0),
                        start=first, stop=last, skip_group_check=True,
                    )

                def out_stage(pr):
                    ytA0, ytA1, ytB0, ytB1 = yts.pop(pr)
                    # ---- normalize + emit [128 rows = 2 heads, 512] ----
                    ystage = op.tile([128, 512], f32, tag="ystage", bufs=2,
                                     name=f"ys{pr}_{I}")
                    rec2 = op.tile([33, 512], f32, tag="rec2",
                                   name=f"rec2{pr}_{I}")
                    sum2 = op.tile([33, 512], f32, tag="sum2",
                                   name=f"sum2{pr}_{I}")
                    sA = op.tile([D + 1, 512], f32, tag="sA", name=f"sA{pr}_{I}")
                    sB = op.tile([D + 1, 512], f32, tag="sB", name=f"sB{pr}_{I}")
                    recA, recB = rec2[0:1, :], rec2[32:33, :]
                    sumA, sumB = sum2[0:1, :], sum2[32:33, :]
                    nc.vector.tensor_copy(sA, ytA1)
                    nc.vector.tensor_copy(sB, ytB1)
                    nc.vector.tensor_add(ystage[0:64, :], ytA0[0:D, :], sA[0:D, :])
                    nc.vector.tensor_add(ystage[64:128, :], ytB0[0:D, :],
                                         sB[0:D, :])
                    nc.vector.tensor_add(sumA, ytA0[D : D + 1, :],
                                         sA[D : D + 1, :])
                    nc.vector.tensor_add(sumB, ytB0[D : D + 1, :],
                                         sB[D : D + 1, :])
                    nc.vector.reciprocal(recA, sumA)
                    nc.vector.reciprocal(recB, sumB)
                    # broadcast 1/sum across partitions via a DRAM bounce
                    # (keeps the PE stream free of output-stage work)
                    recA_d = dp.tile([1, 512], f32, tag="recA_d",
                                     name=f"recAd{pr}_{I}")
                    recB_d = dp.tile([1, 512], f32, tag="recB_d",
                                     name=f"recBd{pr}_{I}")
                    rbc2 = op.tile([128, 512], f32, tag="rbc2",
                                   name=f"rbc2{pr}_{I}")
                    rbcA, rbcB = rbc2[0:64, :], rbc2[64:128, :]
                    nc.sync.dma_start(out=recA_d, in_=recA)
                    nc.sync.dma_start(out=recB_d, in_=recB)
                    nc.sync.dma_start(out=rbcA, in_=_bcast_ap(recA_d, 64))
                    nc.sync.dma_start(out=rbcB, in_=_bcast_ap(recB_d, 64))
                    nc.vector.tensor_mul(ystage[0:64, :], ystage[0:64, :], rbcA)
                    nc.vector.tensor_mul(ystage[64:128, :], ystage[64:128, :],
                                         rbcB)
                    nc.sync.dma_start(
                        out=yT[pr * 128 : (pr + 1) * 128, isl], in_=ystage)

                # 1-stage software pipeline across the whole block: QK(k+1)
                # issues before PV(k) so the PE never sits behind a PV that
                # is waiting on exp
                items = [(pr, J) for pr in range(PAIRS) for J in range(nj)]
                prev = None
                for it in items:
                    if it[1] == 0:
                        alloc_yt(it[0])
                    qk_exp(*it)
                    if prev is not None:
                        pv(*prev)
                        if prev[1] == nj - 1:
                            out_stage(prev[0])
                    prev = it
                pv(*prev)
                out_stage(prev[0])

            # schedule: per t-block, QKV projection then attention I = tb
            # (causal: block I only needs k/v from t-blocks <= I)
            for g in range(12):
                (qkv_group_qk(0, g) if g < 8 else qkv_group_v(0, g - 8))
            for I in range(TB):
                nxt = []
                if I + 1 < TB:
                    load_x(I + 1)
                    nxt = [(qkv_group_qk, I + 1, g) for g in range(8)] + [
                        (qkv_group_v, I + 1, g) for g in range(4)
                    ]
                attn_block(I)
                for fn, a, b in nxt:
                    fn(a, b)
    nc.compile()
    return nc


def _bcast_ap(src_ap, nparts):
    """Partition-broadcast view of a [1, N] DRAM AP -> [nparts, N]."""
    import concourse.bass as bass

    return bass.AP(
        tensor=src_ap.tensor,
        offset=src_ap.offset,
        ap=[[0, nparts]] + list(src_ap.ap)[1:],
    )


def get_nc():
    if "nc" not in _cache:
        _cache["nc"] = _build_nc()
    return _cache["nc"]


def shard_inputs(x, w_attn, b_attn):
    """Full inputs -> per-core input maps (host-side slicing/transposition)."""
    x = np.asarray(x, dtype=np.float32)
    w = np.asarray(w_attn, dtype=np.float32)
    bb = np.asarray(b_attn, dtype=np.float32)
    in_maps = []
    for core in range(N_CORES):
        b, hg = core // 2, core % 2
        r0 = hg * CC  # first q row for this head group
        w_qk = np.ascontiguousarray(
            np.concatenate([w[r0 : r0 + CC, :], w[C + r0 : C + r0 + CC, :]], axis=0).T
        )
        w_v = np.ascontiguousarray(w[2 * C + r0 : 2 * C + r0 + CC, :].T)
        b_qk = np.stack(
            [bb[r0 + ct * 128 : r0 + (ct + 1) * 128] for ct in range(4)]
            + [bb[C + r0 + ct * 128 : C + r0 + (ct + 1) * 128] for ct in range(4)],
            axis=1,
        ).astype(np.float32)
        b_v = bb[2 * C + r0 : 2 * C + r0 + CC].reshape(1, CC).astype(np.float32)
        in_maps.append(
            {
                "xT": np.ascontiguousarray(x[b].T),
                "w_qk": w_qk,
                "w_v": w_v,
                "b_qk": np.ascontiguousarray(b_qk),
                "b_v": np.ascontiguousarray(b_v),
                "ones_d": np.ones((1, 128), dtype=np.float32),
            }
        )
    return in_maps


def run(in_maps, trace=False, **kw):
    from concourse import bass_utils

    nc = get_nc()
    return bass_utils.run_bass_kernel_spmd(
        nc, in_maps, core_ids=list(range(N_CORES)), trace=trace, **kw
    )


def gather_output(results):
    y = np.empty((B, T, E), dtype=np.float32)
    for core in range(N_CORES):
        b, hg = core // 2, core % 2
        y[b, :, hg * CC : (hg + 1) * CC] = results[core]["yT"].T
    return y


def kernel(x, w_attn, b_attn):
    in_maps = shard_inputs(x, w_attn, b_attn)
    res = run(in_maps, trace=False)
    return gather_output(res.results)


# revision 40
# speedup vs baseline: 1.0880x; 1.0481x over previous
"""Causal self-attention (B=4, T=2048, E=1024, H=16) on 8 trn2 NeuronCores.

Sharding: core c -> (batch b = c // 2, head-group hg = c % 2); each core owns
one batch element and 8 of the 16 heads (data parallel on B, tensor parallel
on heads).  No cross-core communication.

Per-core device program (SPMD, same NEFF on all 8 cores), interleaved per
512-token block tb: QKV projection for tb, then attention for query block
I = tb (causal -> only needs k/v from blocks <= tb):
  qT,kT [c,t]-layout (2 heads packed per 128-partition tile), bias on DVE
  v     [t,c]-layout with a ones column per head, bias via K=1 matmul
  attention (all matmuls in 64-row PE tiling mode, no mode switches):
    St[j,i] strip [A|B]: QK row-tile pair computes 2 heads concurrently
    Pt = exp(St/8) on ScalarE (one instr per head pair), causal mask via a
         width-trimmed gpsimd affine_select on diagonal tiles only
    Yt[d|sum, i]: PV row-tile pair (j split 64+64) -> 2 psum partials,
         summed on DVE; softmax denominators come out as row 64
    y = Yt[:64] * (1/Yt[64]); the broadcast of the reciprocal across
        partitions is a K=1 matmul (ones^T @ recip)
Output written as yT [c, t]; the host transposes and concatenates.
"""

import sys

sys.path.insert(0, "/opt/trn_rl_repo")

import numpy as np

N_CORES = 8
B, T, E = 4, 2048, 1024
H, D = 16, 64
C = E                 # q/k/v channel count (4th qkv chunk unused)
HPC = H // 2          # heads per core
CC = HPC * D          # per-core channels = 512
ES = E // 128         # 8 e-tiles (contraction)
TB = T // 512         # 4 t/i blocks of 512
NJ = T // 128         # 16 j-tiles of 128
PAIRS = HPC // 2      # 4 head pairs per core

_cache = {}


def _build_nc():
    import concourse.mybir as mybir
    import concourse.tile as tile
    from concourse import bacc

    f32 = mybir.dt.float32
    f32r = mybir.dt.float32r
    Act = mybir.ActivationFunctionType
    is_ge = mybir.AluOpType.is_ge

    nc = bacc.Bacc("TRN2", target_bir_lowering=False, debug=False)

    xT = nc.dram_tensor("xT", [E, T], f32r, kind="ExternalInput").ap()
    w_qk = nc.dram_tensor("w_qk", [E, 2 * CC], f32r, kind="ExternalInput").ap()
    w_v = nc.dram_tensor("w_v", [E, CC], f32r, kind="ExternalInput").ap()
    b_qk = nc.dram_tensor("b_qk", [128, 8], f32, kind="ExternalInput").ap()
    b_v = nc.dram_tensor("b_v", [1, CC], f32r, kind="ExternalInput").ap()
    ones_d = nc.dram_tensor("ones_d", [1, 128], f32r, kind="ExternalInput").ap()
    yT = nc.dram_tensor("yT", [CC, T], f32, kind="ExternalOutput").ap()

    with tile.TileContext(nc) as tc:
        with (
            tc.tile_pool(name="persist", bufs=1) as pp,
            tc.tile_pool(name="psum", bufs=1, space="PSUM") as psp,
            tc.tile_pool(name="xpool", bufs=2) as xp,
            tc.tile_pool(name="ptpool", bufs=4) as ptp,
            tc.tile_pool(name="opool", bufs=1) as op,
            tc.tile_pool(name="dpool", bufs=2, space="DRAM") as dp,
        ):
            # ---- persistent SBUF state ----
            qk_sb = [pp.tile([128, T], f32r, name=f"qk{ct}") for ct in range(8)]
            # v plus a ones column per head: [t-part, head, t-tile, 65]
            v1_sb = pp.tile([128, HPC, NJ, D + 1], f32r, name="v1")
            bqk_sb = pp.tile([128, 8], f32, name="bqk")
            bv_sb = pp.tile([1, CC], f32r, name="bv")
            ones_sb = pp.tile([1, 128], f32r, name="ones")
            wqk_t = []
            wv_t = []

            # input DMAs: x(tb0) first so the first matmul group can start,
            # then weights, then the small vectors
            xs_tb = {}

            def load_x(tb):
                tsl = slice(tb * 512, (tb + 1) * 512)
                xs = []
                for e in range(ES):
                    xe = xp.tile([128, 512], f32r, tag=f"x{e}",
                                 bufs=(2 if e < 3 else 1), name=f"x{e}_{tb}")
                    nc.sync.dma_start(out=xe, in_=xT[e * 128 : (e + 1) * 128, tsl])
                    xs.append(xe)
                xs_tb[tb] = xs

            # small constants first, then x/w interleaved per e-tile so the
            # first matmul accumulation group can finish as early as possible
            nc.sync.dma_start(out=bqk_sb, in_=b_qk)
            nc.sync.dma_start(out=bv_sb, in_=b_v)
            nc.sync.dma_start(out=ones_sb, in_=ones_d)
            tsl0 = slice(0, 512)
            xs0 = []
            for e in range(ES):
                xe = xp.tile([128, 512], f32r, tag=f"x{e}",
                             bufs=(2 if e < 3 else 1), name=f"x{e}_0")
                nc.sync.dma_start(out=xe, in_=xT[e * 128 : (e + 1) * 128, tsl0])
                xs0.append(xe)
                wqk = pp.tile([128, 2 * CC], f32r, name=f"wqk{e}")
                nc.sync.dma_start(out=wqk, in_=w_qk[e * 128 : (e + 1) * 128, :])
                wqk_t.append(wqk)
            xs_tb[0] = xs0
            for e in range(ES):
                wv = pp.tile([128, CC], f32r, name=f"wv{e}")
                nc.sync.dma_start(out=wv, in_=w_v[e * 128 : (e + 1) * 128, :])
                wv_t.append(wv)
            ones_bc = _bcast_ap(ones_d, 128)
            nc.sync.dma_start(out=v1_sb[:, :, :, D : D + 1], in_=ones_bc)

            def qkv_group_qk(tb, ct):
                tsl = slice(tb * 512, (tb + 1) * 512)
                xs = xs_tb[tb]
                ps = psp.tile([128, 512], f32, tag="st", bufs=2,
                              name=f"psqk{ct}_{tb}")
                for e in range(ES):
                    nc.tensor.matmul(
                        ps,
                        wqk_t[e][:, ct * 128 : (ct + 1) * 128],
                        xs[e],
                        start=(e == 0),
                        stop=(e == ES - 1),
                    )
                nc.scalar.activation(
                    qk_sb[ct][:, tsl], ps, Act.Identity,
                    bias=bqk_sb[:, ct : ct + 1], scale=1.0)

            def qkv_group_v(tb, k4):
                xs = xs_tb[tb]
                tt = tb * 4 + k4
                psv = psp.tile([128, 512], f32, tag="st", bufs=2,
                               name=f"psv{tt}")
                nc.tensor.matmul(
                    psv, ones_sb, bv_sb,
                    start=True, stop=False, skip_group_check=True,
                )
                for e in range(ES):
                    nc.tensor.matmul(
                        psv,
                        xs[e][:, k4 * 128 : (k4 + 1) * 128],
                        wv_t[e],
                        start=False,
                        stop=(e == ES - 1),
                        skip_group_check=True,
                    )
                nc.vector.tensor_copy(
                    v1_sb[:, :, tt, 0:D],
                    psv.rearrange("p (h d) -> p h d", d=D),
                )

            def attn_block(I, nxt=()):
                isl = slice(I * 512, (I + 1) * 512)
                nj = 4 * I + 4  # causal j-tiles for this i-block
                yts = {}
                pts = {}

                def alloc_yt(pr):
                    yts[pr] = [
                        psp.tile([D + 1, 512], f32, tag=f"yt{n}",
                                 name=f"yt{n}_{pr}_{I}")
                        for n in ("A0", "A1", "B0", "B1")
                    ]

                def qk_exp(pr, J):
                    qt = qk_sb[pr]
                    kt = qk_sb[4 + pr]
                    jsl = slice(J * 128, (J + 1) * 128)
                    st = psp.tile([128, 1024], f32, tag="st", bufs=2,
                                  name=f"st{pr}_{I}_{J}")
                    # QK row-tile pair: head A rows 0-63, head B 64-127
                    nc.tensor.matmul(
                        st[:, 0:512], kt[0:64, jsl], qt[0:64, isl],
                        tile_position=(0, 0),
                    )
                    nc.tensor.matmul(
                        st[:, 512:1024], kt[64:128, jsl], qt[64:128, isl],
                        tile_position=(64, 0),
                    )
                    pt = ptp.tile([128, 1024], f32r, tag="pt",
                                  name=f"pt{pr}_{I}_{J}")
                    nc.scalar.activation(pt, st, Act.Exp, scale=0.125)
                    r = J - 4 * I
                    if r >= 0:  # diagonal tile: causal mask, trimmed width
                        w = (r + 1) * 128
                        for off in (0, 512):
                            # keep where (512I + y) - (128J + x) >= 0
                            nc.gpsimd.affine_select(
                                out=pt[:, off : off + w],
                                in_=pt[:, off : off + w],
                                compare_op=is_ge,
                                fill=0.0,
                                base=-128 * r,
                                pattern=[[1, w]],
                                channel_multiplier=-1,
                            )
                    pts[(pr, J)] = pt

                def pv(pr, J):
                    pt = pts.pop((pr, J))
                    ytA0, ytA1, ytB0, ytB1 = yts[pr]
                    first, last = (J == 0), (J == nj - 1)
                    # PV row-tile pairs (j contraction split 64+64)
                    nc.tensor.matmul(
                        ytA0, v1_sb[0:64, 2 * pr, J, :], pt[0:64, 0:512],
                        tile_position=(0, 0),
                        start=first, stop=last, skip_group_check=True,
                    )
                    nc.tensor.matmul(
                        ytA1, v1_sb[64:128, 2 * pr, J, :], pt[64:128, 0:512],
                        tile_position=(64, 0),
                        start=first, stop=last, skip_group_check=True,
                    )
                    nc.tensor.matmul(
                        ytB0, v1_sb[0:64, 2 * pr + 1, J, :], pt[0:64, 512:1024],
                        tile_position=(0, 0),
                        start=first, stop=last, skip_group_check=True,
                    )
                    nc.tensor.matmul(
                        ytB1, v1_sb[64:128, 2 * pr + 1, J, :],
                        pt[64:128, 512:1024],
                        tile_position=(64, 0),
                        start=first, stop=last, skip_group_check=True,
                    )

                def out_stage(pr):
                    ytA0, ytA1, ytB0, ytB1 = yts.pop(pr)
                    # ---- normalize + emit [128 rows = 2 heads, 512] ----
                    # copy + single add per head frees the yt psum slots after
                    # only two DVE ops; row 64 of tmp holds the softmax sum
                    ystage = op.tile([128, 512], f32, tag="ystage", bufs=2,
                                     name=f"ys{pr}_{I}")
                    rec2 = op.tile([33, 512], f32, tag="rec2",
                                   name=f"rec2{pr}_{I}")
                    tmpA = op.tile([D + 1, 512], f32, tag="tmpA",
                                   name=f"tmpA{pr}_{I}")
                    sum1 = op.tile([1, 512], f32, tag="sum1",
                                   name=f"sum1{pr}_{I}")
                    sA = op.tile([D + 1, 512], f32, tag="sA", name=f"sA{pr}_{I}")
                    sB = op.tile([D + 1, 512], f32, tag="sB", name=f"sB{pr}_{I}")
                    recA, recB = rec2[0:1, :], rec2[32:33, :]
                    # head A: combine into tmpA (base 0 throughout)
                    nc.vector.tensor_copy(sA, ytA1)
                    nc.vector.tensor_add(tmpA, ytA0, sA)
                    nc.vector.reciprocal(recA, tmpA[D : D + 1, :])
                    # head B: combine straight into ystage rows 64-127 so the
                    # final in-place mul keeps matching base partitions
                    nc.vector.tensor_copy(sB, ytB1)
                    nc.vector.tensor_add(ystage[64:128, :], ytB0[0:D, :],
                                         sB[0:D, :])
                    nc.vector.tensor_add(sum1, ytB0[D : D + 1, :],
                                         sB[D : D + 1, :])
                    nc.vector.reciprocal(recB, sum1)
                    # broadcast 1/sum across partitions via a DRAM bounce
                    # (keeps the PE stream free of output-stage work)
                    recA_d = dp.tile([1, 512], f32, tag="recA_d",
                                     name=f"recAd{pr}_{I}")
                    recB_d = dp.tile([1, 512], f32, tag="recB_d",
                                     name=f"recBd{pr}_{I}")
                    rbc2 = op.tile([128, 512], f32, tag="rbc2",
                                   name=f"rbc2{pr}_{I}")
                    rbcA, rbcB = rbc2[0:64, :], rbc2[64:128, :]
                    nc.sync.dma_start(out=recA_d, in_=recA)
                    nc.sync.dma_start(out=recB_d, in_=recB)
                    nc.sync.dma_start(out=rbcA, in_=_bcast_ap(recA_d, 64))
                    nc.sync.dma_start(out=rbcB, in_=_bcast_ap(recB_d, 64))
                    nc.vector.tensor_mul(ystage[0:64, :], tmpA[0:D, :], rbcA)
                    nc.vector.tensor_mul(ystage[64:128, :], ystage[64:128, :],
                                         rbcB)
                    nc.sync.dma_start(
                        out=yT[pr * 128 : (pr + 1) * 128, isl], in_=ystage)

                # 1-stage software pipeline across the whole block: QK(k+1)
                # issues before PV(k) so the PE never sits behind a PV that
                # is waiting on exp
                items = [(pr, J) for pr in range(PAIRS) for J in range(nj)]
                emitted = 0
                done = 0

                def emit_qk(k):
                    pr, J = items[k]
                    if J == 0:
                        alloc_yt(pr)
                    qk_exp(pr, J)

                # lookahead-1 pipeline, deepened to 2 at pair boundaries so
                # the first PV of a new pair isn't reached while the DVE is
                # still releasing the previous pair's yt slots
                nxt = list(nxt)
                stride = max(1, len(items) // len(nxt)) if nxt else 0
                emit_qk(0)
                emitted = 1
                while done < len(items):
                    if emitted < len(items):
                        emit_qk(emitted)
                        if items[emitted][1] == 0 and emitted + 1 < len(items):
                            emit_qk(emitted + 1)
                            emitted += 1
                        emitted += 1
                    pr, J = items[done]
                    pv(pr, J)
                    if J == nj - 1:
                        out_stage(pr)
                    done += 1
                    # weave next t-block's QKV groups into the PE stream
                    if nxt and done % stride == 0:
                        fn, a, b = nxt.pop(0)
                        fn(a, b)
                for fn, a, b in nxt:
                    fn(a, b)

            # schedule: per t-block, QKV projection then attention I = tb
            # (causal: block I only needs k/v from t-blocks <= I)
            for g in range(12):
                (qkv_group_qk(0, g) if g < 8 else qkv_group_v(0, g - 8))
            for I in range(TB):
                nxt = []
                if I + 1 < TB:
                    load_x(I + 1)
                    nxt = [(qkv_group_qk, I + 1, g) for g in range(8)] + [
                        (qkv_group_v, I + 1, g) for g in range(4)
                    ]
                attn_block(I, nxt)
    nc.compile()
    return nc


def _bcast_ap(src_ap, nparts):
    """Partition-broadcast view of a [1, N] DRAM AP -> [nparts, N]."""
    import concourse.bass as bass

    return bass.AP(
        tensor=src_ap.tensor,
        offset=src_ap.offset,
        ap=[[0, nparts]] + list(src_ap.ap)[1:],
    )


def get_nc():
    if "nc" not in _cache:
        _cache["nc"] = _build_nc()
    return _cache["nc"]


def shard_inputs(x, w_attn, b_attn):
    """Full inputs -> per-core input maps (host-side slicing/transposition)."""
    x = np.asarray(x, dtype=np.float32)
    w = np.asarray(w_attn, dtype=np.float32)
    bb = np.asarray(b_attn, dtype=np.float32)
    in_maps = []
    for core in range(N_CORES):
        b, hg = core // 2, core % 2
        r0 = hg * CC  # first q row for this head group
        w_qk = np.ascontiguousarray(
            np.concatenate([w[r0 : r0 + CC, :], w[C + r0 : C + r0 + CC, :]], axis=0).T
        )
        w_v = np.ascontiguousarray(w[2 * C + r0 : 2 * C + r0 + CC, :].T)
        b_qk = np.stack(
            [bb[r0 + ct * 128 : r0 + (ct + 1) * 128] for ct in range(4)]
            + [bb[C + r0 + ct * 128 : C + r0 + (ct + 1) * 128] for ct in range(4)],
            axis=1,
        ).astype(np.float32)
        b_v = bb[2 * C + r0 : 2 * C + r0 + CC].reshape(1, CC).astype(np.float32)
        in_maps.append(
            {
                "xT": np.ascontiguousarray(x[b].T),
                "w_qk": w_qk,
                "w_v": w_v,
                "b_qk": np.ascontiguousarray(b_qk),
                "b_v": np.ascontiguousarray(b_v),
                "ones_d": np.ones((1, 128), dtype=np.float32),
            }
        )
    return in_maps


def run(in_maps, trace=False, **kw):
    from concourse import bass_utils

    nc = get_nc()
    return bass_utils.run_bass_kernel_spmd(
        nc, in_maps, core_ids=list(range(N_CORES)), trace=trace, **kw
    )


def gather_output(results):
    y = np.empty((B, T, E), dtype=np.float32)
    for core in range(N_CORES):
        b, hg = core // 2, core % 2
        y[b, :, hg * CC : (hg + 1) * CC] = results[core]["yT"].T
    return y


def kernel(x, w_attn, b_attn):
    in_maps = shard_inputs(x, w_attn, b_attn)
    res = run(in_maps, trace=False)
    return gather_output(res.results)


# revision 42
# speedup vs baseline: 1.1142x; 1.0240x over previous
"""Causal self-attention (B=4, T=2048, E=1024, H=16) on 8 trn2 NeuronCores.

Sharding: core c -> (batch b = c // 2, head-group hg = c % 2); each core owns
one batch element and 8 of the 16 heads (data parallel on B, tensor parallel
on heads).  No cross-core communication.

Per-core device program (SPMD, same NEFF on all 8 cores), interleaved per
512-token block tb: QKV projection for tb, then attention for query block
I = tb (causal -> only needs k/v from blocks <= tb):
  qT,kT [c,t]-layout (2 heads packed per 128-partition tile), bias on DVE
  v     [t,c]-layout with a ones column per head, bias via K=1 matmul
  attention (all matmuls in 64-row PE tiling mode, no mode switches):
    St[j,i] strip [A|B]: QK row-tile pair computes 2 heads concurrently
    Pt = exp(St/8) on ScalarE (one instr per head pair), causal mask via a
         width-trimmed gpsimd affine_select on diagonal tiles only
    Yt[d|sum, i]: PV row-tile pair (j split 64+64) -> 2 psum partials,
         summed on DVE; softmax denominators come out as row 64
    y = Yt[:64] * (1/Yt[64]); the broadcast of the reciprocal across
        partitions is a K=1 matmul (ones^T @ recip)
Output written as yT [c, t]; the host transposes and concatenates.
"""

import sys

sys.path.insert(0, "/opt/trn_rl_repo")

import numpy as np

N_CORES = 8
B, T, E = 4, 2048, 1024
H, D = 16, 64
C = E                 # q/k/v channel count (4th qkv chunk unused)
HPC = H // 2          # heads per core
CC = HPC * D          # per-core channels = 512
ES = E // 128         # 8 e-tiles (contraction)
TB = T // 512         # 4 t/i blocks of 512
NJ = T // 128         # 16 j-tiles of 128
PAIRS = HPC // 2      # 4 head pairs per core

_cache = {}


def _build_nc():
    import concourse.mybir as mybir
    import concourse.tile as tile
    from concourse import bacc

    f32 = mybir.dt.float32
    f32r = mybir.dt.float32r
    Act = mybir.ActivationFunctionType
    is_ge = mybir.AluOpType.is_ge

    nc = bacc.Bacc("TRN2", target_bir_lowering=False, debug=False)

    xT = nc.dram_tensor("xT", [E, T], f32r, kind="ExternalInput").ap()
    w_qk = nc.dram_tensor("w_qk", [E, 2 * CC], f32r, kind="ExternalInput").ap()
    w_v = nc.dram_tensor("w_v", [E, CC], f32r, kind="ExternalInput").ap()
    b_qk = nc.dram_tensor("b_qk", [128, 8], f32, kind="ExternalInput").ap()
    b_v = nc.dram_tensor("b_v", [1, CC], f32r, kind="ExternalInput").ap()
    ones_d = nc.dram_tensor("ones_d", [1, 128], f32r, kind="ExternalInput").ap()
    yT = nc.dram_tensor("yT", [CC, T], f32, kind="ExternalOutput").ap()

    with tile.TileContext(nc) as tc:
        with (
            tc.tile_pool(name="persist", bufs=1) as pp,
            tc.tile_pool(name="psum", bufs=1, space="PSUM") as psp,
            tc.tile_pool(name="xpool", bufs=2) as xp,
            tc.tile_pool(name="ptpool", bufs=4) as ptp,
            tc.tile_pool(name="opool", bufs=1) as op,
            tc.tile_pool(name="dpool", bufs=2, space="DRAM") as dp,
        ):
            # ---- persistent SBUF state ----
            qk_sb = [pp.tile([128, T], f32r, name=f"qk{ct}") for ct in range(8)]
            # v plus a ones column per head: [t-part, head, t-tile, 65]
            v1_sb = pp.tile([128, HPC, NJ, D + 1], f32r, name="v1")
            bqk_sb = pp.tile([128, 8], f32, name="bqk")
            bv_sb = pp.tile([1, CC], f32r, name="bv")
            ones_sb = pp.tile([1, 128], f32r, name="ones")
            wqk_t = []
            wv_t = []

            # input DMAs: x(tb0) first so the first matmul group can start,
            # then weights, then the small vectors
            xs_tb = {}

            def load_x(tb):
                tsl = slice(tb * 512, (tb + 1) * 512)
                xs = []
                for e in range(ES):
                    xe = xp.tile([128, 512], f32r, tag=f"x{e}",
                                 bufs=(2 if e < 3 else 1), name=f"x{e}_{tb}")
                    nc.sync.dma_start(out=xe, in_=xT[e * 128 : (e + 1) * 128, tsl])
                    xs.append(xe)
                xs_tb[tb] = xs

            # small constants first, then x/w interleaved per e-tile so the
            # first matmul accumulation group can finish as early as possible
            nc.sync.dma_start(out=bqk_sb, in_=b_qk)
            nc.sync.dma_start(out=bv_sb, in_=b_v)
            nc.sync.dma_start(out=ones_sb, in_=ones_d)
            tsl0 = slice(0, 512)
            xs0 = []
            for e in range(ES):
                xe = xp.tile([128, 512], f32r, tag=f"x{e}",
                             bufs=(2 if e < 3 else 1), name=f"x{e}_0")
                nc.sync.dma_start(out=xe, in_=xT[e * 128 : (e + 1) * 128, tsl0])
                xs0.append(xe)
                wqk = pp.tile([128, 2 * CC], f32r, name=f"wqk{e}")
                nc.sync.dma_start(out=wqk, in_=w_qk[e * 128 : (e + 1) * 128, :])
                wqk_t.append(wqk)
            xs_tb[0] = xs0
            for e in range(ES):
                wv = pp.tile([128, CC], f32r, name=f"wv{e}")
                nc.sync.dma_start(out=wv, in_=w_v[e * 128 : (e + 1) * 128, :])
                wv_t.append(wv)
            ones_bc = _bcast_ap(ones_d, 128)
            nc.sync.dma_start(out=v1_sb[:, :, :, D : D + 1], in_=ones_bc)

            def qkv_group_qk(tb, ct):
                tsl = slice(tb * 512, (tb + 1) * 512)
                xs = xs_tb[tb]
                ps = psp.tile([128, 512], f32, tag="st", bufs=2,
                              name=f"psqk{ct}_{tb}")
                for e in range(ES):
                    nc.tensor.matmul(
                        ps,
                        wqk_t[e][:, ct * 128 : (ct + 1) * 128],
                        xs[e],
                        start=(e == 0),
                        stop=(e == ES - 1),
                    )
                nc.scalar.activation(
                    qk_sb[ct][:, tsl], ps, Act.Identity,
                    bias=bqk_sb[:, ct : ct + 1], scale=1.0)

            def qkv_group_v(tb, k4):
                xs = xs_tb[tb]
                tt = tb * 4 + k4
                psv = psp.tile([128, 512], f32, tag="st", bufs=2,
                               name=f"psv{tt}")
                nc.tensor.matmul(
                    psv, ones_sb, bv_sb,
                    start=True, stop=False, skip_group_check=True,
                )
                for e in range(ES):
                    nc.tensor.matmul(
                        psv,
                        xs[e][:, k4 * 128 : (k4 + 1) * 128],
                        wv_t[e],
                        start=False,
                        stop=(e == ES - 1),
                        skip_group_check=True,
                    )
                nc.vector.tensor_copy(
                    v1_sb[:, :, tt, 0:D],
                    psv.rearrange("p (h d) -> p h d", d=D),
                )

            def attn_block(I, nxt=()):
                isl = slice(I * 512, (I + 1) * 512)
                nj = 4 * I + 4  # causal j-tiles for this i-block
                yts = {}
                pts = {}

                def alloc_yt(pr):
                    yts[pr] = [
                        psp.tile([D + 1, 512], f32, tag=f"yt{n}",
                                 name=f"yt{n}_{pr}_{I}")
                        for n in ("A0", "A1", "B0", "B1")
                    ]

                def qk_exp(pr, J):
                    qt = qk_sb[pr]
                    kt = qk_sb[4 + pr]
                    jsl = slice(J * 128, (J + 1) * 128)
                    st = psp.tile([128, 1024], f32, tag="st", bufs=2,
                                  name=f"st{pr}_{I}_{J}")
                    # QK row-tile pair: head A rows 0-63, head B 64-127
                    nc.tensor.matmul(
                        st[:, 0:512], kt[0:64, jsl], qt[0:64, isl],
                        tile_position=(0, 0),
                    )
                    nc.tensor.matmul(
                        st[:, 512:1024], kt[64:128, jsl], qt[64:128, isl],
                        tile_position=(64, 0),
                    )
                    pt = ptp.tile([128, 1024], f32r, tag="pt",
                                  name=f"pt{pr}_{I}_{J}")
                    nc.scalar.activation(pt, st, Act.Exp, scale=0.125)
                    r = J - 4 * I
                    if r >= 0:  # diagonal tile: causal mask, trimmed width
                        w = (r + 1) * 128
                        for off in (0, 512):
                            # keep where (512I + y) - (128J + x) >= 0
                            nc.gpsimd.affine_select(
                                out=pt[:, off : off + w],
                                in_=pt[:, off : off + w],
                                compare_op=is_ge,
                                fill=0.0,
                                base=-128 * r,
                                pattern=[[1, w]],
                                channel_multiplier=-1,
                            )
                    pts[(pr, J)] = pt

                def pv(pr, J):
                    pt = pts.pop((pr, J))
                    ytA0, ytA1, ytB0, ytB1 = yts[pr]
                    first, last = (J == 0), (J == nj - 1)
                    # PV row-tile pairs (j contraction split 64+64)
                    nc.tensor.matmul(
                        ytA0, v1_sb[0:64, 2 * pr, J, :], pt[0:64, 0:512],
                        tile_position=(0, 0),
                        start=first, stop=last, skip_group_check=True,
                    )
                    nc.tensor.matmul(
                        ytA1, v1_sb[64:128, 2 * pr, J, :], pt[64:128, 0:512],
                        tile_position=(64, 0),
                        start=first, stop=last, skip_group_check=True,
                    )
                    nc.tensor.matmul(
                        ytB0, v1_sb[0:64, 2 * pr + 1, J, :], pt[0:64, 512:1024],
                        tile_position=(0, 0),
                        start=first, stop=last, skip_group_check=True,
                    )
                    nc.tensor.matmul(
                        ytB1, v1_sb[64:128, 2 * pr + 1, J, :],
                        pt[64:128, 512:1024],
                        tile_position=(64, 0),
                        start=first, stop=last, skip_group_check=True,
                    )

                def out_stage(pr):
                    ytA0, ytA1, ytB0, ytB1 = yts.pop(pr)
                    # ---- normalize + emit [128 rows = 2 heads, 512] ----
                    # copy + single add per head frees the yt psum slots after
                    # only two DVE ops; row 64 of tmp holds the softmax sum
                    ystage = op.tile([128, 512], f32, tag="ystage", bufs=2,
                                     name=f"ys{pr}_{I}")
                    rec2 = op.tile([33, 512], f32, tag="rec2",
                                   name=f"rec2{pr}_{I}")
                    tmpA = op.tile([D + 1, 512], f32, tag="tmpA",
                                   name=f"tmpA{pr}_{I}")
                    sum1 = op.tile([1, 512], f32, tag="sum1",
                                   name=f"sum1{pr}_{I}")
                    sA = op.tile([D + 1, 512], f32, tag="sA", name=f"sA{pr}_{I}")
                    sB = op.tile([D + 1, 512], f32, tag="sB", name=f"sB{pr}_{I}")
                    recA, recB = rec2[0:1, :], rec2[32:33, :]
                    # head A: combine into tmpA (base 0 throughout)
                    nc.vector.tensor_copy(sA, ytA1)
                    nc.vector.tensor_add(tmpA, ytA0, sA)
                    nc.vector.reciprocal(recA, tmpA[D : D + 1, :])
                    # head B: combine straight into ystage rows 64-127 so the
                    # final in-place mul keeps matching base partitions
                    nc.vector.tensor_copy(sB, ytB1)
                    nc.vector.tensor_add(ystage[64:128, :], ytB0[0:D, :],
                                         sB[0:D, :])
                    nc.vector.tensor_add(sum1, ytB0[D : D + 1, :],
                                         sB[D : D + 1, :])
                    nc.vector.reciprocal(recB, sum1)
                    # broadcast 1/sum across partitions via a DRAM bounce
                    # (keeps the PE stream free of output-stage work)
                    recA_d = dp.tile([1, 512], f32, tag="recA_d",
                                     name=f"recAd{pr}_{I}")
                    recB_d = dp.tile([1, 512], f32, tag="recB_d",
                                     name=f"recBd{pr}_{I}")
                    rbc2 = op.tile([128, 512], f32, tag="rbc2",
                                   name=f"rbc2{pr}_{I}")
                    rbcA, rbcB = rbc2[0:64, :], rbc2[64:128, :]
                    nc.sync.dma_start(out=recA_d, in_=recA)
                    nc.sync.dma_start(out=recB_d, in_=recB)
                    nc.sync.dma_start(out=rbcA, in_=_bcast_ap(recA_d, 64))
                    nc.sync.dma_start(out=rbcB, in_=_bcast_ap(recB_d, 64))
                    nc.vector.tensor_mul(ystage[0:64, :], tmpA[0:D, :], rbcA)
                    nc.vector.tensor_mul(ystage[64:128, :], ystage[64:128, :],
                                         rbcB)
                    nc.sync.dma_start(
                        out=yT[pr * 128 : (pr + 1) * 128, isl], in_=ystage)

                # 1-stage software pipeline across the whole block: QK(k+1)
                # issues before PV(k) so the PE never sits behind a PV that
                # is waiting on exp
                items = [(pr, J) for pr in range(PAIRS) for J in range(nj)]
                emitted = 0
                done = 0

                def emit_qk(k):
                    pr, J = items[k]
                    if J == 0:
                        alloc_yt(pr)
                    qk_exp(pr, J)

                # lookahead-1 pipeline, deepened to 2 at pair boundaries so
                # the first PV of a new pair isn't reached while the DVE is
                # still releasing the previous pair's yt slots
                nxt = list(nxt)
                stride = max(1, len(items) // len(nxt)) if nxt else 0
                emitted = 0
                # constant lookahead-2 pipeline: QK/exp for items k+1 and k+2
                # are already in the stream when PV(k) issues, so neither a
                # PV wait nor a woven QKV group ever starves ScalarE
                for k in range(len(items)):
                    while emitted < min(k + 4, len(items)):
                        emit_qk(emitted)
                        emitted += 1
                    pr, J = items[k]
                    pv(pr, J)
                    if J == nj - 1:
                        out_stage(pr)
                    # weave next t-block's QKV groups into the PE stream
                    if nxt and (k + 1) % stride == 0:
                        fn, a, b = nxt.pop(0)
                        fn(a, b)
                for fn, a, b in nxt:
                    fn(a, b)

            # schedule: per t-block, QKV projection then attention I = tb
            # (causal: block I only needs k/v from t-blocks <= I)
            for g in range(12):
                (qkv_group_qk(0, g) if g < 8 else qkv_group_v(0, g - 8))
            for I in range(TB):
                nxt = []
                if I + 1 < TB:
                    load_x(I + 1)
                    nxt = [(qkv_group_qk, I + 1, g) for g in range(8)] + [
                        (qkv_group_v, I + 1, g) for g in range(4)
                    ]
                attn_block(I, nxt)
    nc.compile()
    return nc


def _bcast_ap(src_ap, nparts):
    """Partition-broadcast view of a [1, N] DRAM AP -> [nparts, N]."""
    import concourse.bass as bass

    return bass.AP(
        tensor=src_ap.tensor,
        offset=src_ap.offset,
        ap=[[0, nparts]] + list(src_ap.ap)[1:],
    )


def get_nc():
    if "nc" not in _cache:
        _cache["nc"] = _build_nc()
    return _cache["nc"]


def shard_inputs(x, w_attn, b_attn):
    """Full inputs -> per-core input maps (host-side slicing/transposition)."""
    x = np.asarray(x, dtype=np.float32)
    w = np.asarray(w_attn, dtype=np.float32)
    bb = np.asarray(b_attn, dtype=np.float32)
    in_maps = []
    for core in range(N_CORES):
        b, hg = core // 2, core % 2
        r0 = hg * CC  # first q row for this head group
        w_qk = np.ascontiguousarray(
            np.concatenate([w[r0 : r0 + CC, :], w[C + r0 : C + r0 + CC, :]], axis=0).T
        )
        w_v = np.ascontiguousarray(w[2 * C + r0 : 2 * C + r0 + CC, :].T)
        b_qk = np.stack(
            [bb[r0 + ct * 128 : r0 + (ct + 1) * 128] for ct in range(4)]
            + [bb[C + r0 + ct * 128 : C + r0 + (ct + 1) * 128] for ct in range(4)],
            axis=1,
        ).astype(np.float32)
        b_v = bb[2 * C + r0 : 2 * C + r0 + CC].reshape(1, CC).astype(np.float32)
        in_maps.append(
            {
                "xT": np.ascontiguousarray(x[b].T),
                "w_qk": w_qk,
                "w_v": w_v,
                "b_qk": np.ascontiguousarray(b_qk),
                "b_v": np.ascontiguousarray(b_v),
                "ones_d": np.ones((1, 128), dtype=np.float32),
            }
        )
    return in_maps


def run(in_maps, trace=False, **kw):
    from concourse import bass_utils

    nc = get_nc()
    return bass_utils.run_bass_kernel_spmd(
        nc, in_maps, core_ids=list(range(N_CORES)), trace=trace, **kw
    )


def gather_output(results):
    y = np.empty((B, T, E), dtype=np.float32)
    for core in range(N_CORES):
        b, hg = core // 2, core % 2
        y[b, :, hg * CC : (hg + 1) * CC] = results[core]["yT"].T
    return y


def kernel(x, w_attn, b_attn):
    in_maps = shard_inputs(x, w_attn, b_attn)
    res = run(in_maps, trace=False)
    return gather_output(res.results)


# revision 43
# speedup vs baseline: 1.1197x; 1.0049x over previous
"""Causal self-attention (B=4, T=2048, E=1024, H=16) on 8 trn2 NeuronCores.

Sharding: core c -> (batch b = c // 2, head-group hg = c % 2); each core owns
one batch element and 8 of the 16 heads (data parallel on B, tensor parallel
on heads).  No cross-core communication.

Per-core device program (SPMD, same NEFF on all 8 cores), interleaved per
512-token block tb: QKV projection for tb, then attention for query block
I = tb (causal -> only needs k/v from blocks <= tb):
  qT,kT [c,t]-layout (2 heads packed per 128-partition tile), bias on DVE
  v     [t,c]-layout with a ones column per head, bias via K=1 matmul
  attention (all matmuls in 64-row PE tiling mode, no mode switches):
    St[j,i] strip [A|B]: QK row-tile pair computes 2 heads concurrently
    Pt = exp(St/8) on ScalarE (one instr per head pair), causal mask via a
         width-trimmed gpsimd affine_select on diagonal tiles only
    Yt[d|sum, i]: PV row-tile pair (j split 64+64) -> 2 psum partials,
         summed on DVE; softmax denominators come out as row 64
    y = Yt[:64] * (1/Yt[64]); the broadcast of the reciprocal across
        partitions is a K=1 matmul (ones^T @ recip)
Output written as yT [c, t]; the host transposes and concatenates.
"""

import sys

sys.path.insert(0, "/opt/trn_rl_repo")

import numpy as np

N_CORES = 8
B, T, E = 4, 2048, 1024
H, D = 16, 64
C = E                 # q/k/v channel count (4th qkv chunk unused)
HPC = H // 2          # heads per core
CC = HPC * D          # per-core channels = 512
ES = E // 128         # 8 e-tiles (contraction)
TB = T // 512         # 4 t/i blocks of 512
NJ = T // 128         # 16 j-tiles of 128
PAIRS = HPC // 2      # 4 head pairs per core

_cache = {}


def _build_nc():
    import concourse.mybir as mybir
    import concourse.tile as tile
    from concourse import bacc

    f32 = mybir.dt.float32
    f32r = mybir.dt.float32r
    Act = mybir.ActivationFunctionType
    is_ge = mybir.AluOpType.is_ge

    nc = bacc.Bacc("TRN2", target_bir_lowering=False, debug=False)

    xT = nc.dram_tensor("xT", [E, T], f32r, kind="ExternalInput").ap()
    w_qk = nc.dram_tensor("w_qk", [E, 2 * CC], f32r, kind="ExternalInput").ap()
    w_v = nc.dram_tensor("w_v", [E, CC], f32r, kind="ExternalInput").ap()
    b_qk = nc.dram_tensor("b_qk", [128, 8], f32, kind="ExternalInput").ap()
    b_v = nc.dram_tensor("b_v", [1, CC], f32r, kind="ExternalInput").ap()
    ones_d = nc.dram_tensor("ones_d", [1, 128], f32r, kind="ExternalInput").ap()
    yT = nc.dram_tensor("yT", [CC, T], f32, kind="ExternalOutput").ap()

    with tile.TileContext(nc) as tc:
        with (
            tc.tile_pool(name="persist", bufs=1) as pp,
            tc.tile_pool(name="psum", bufs=1, space="PSUM") as psp,
            tc.tile_pool(name="xpool", bufs=2) as xp,
            tc.tile_pool(name="ptpool", bufs=4) as ptp,
            tc.tile_pool(name="opool", bufs=1) as op,
            tc.tile_pool(name="dpool", bufs=2, space="DRAM") as dp,
        ):
            # ---- persistent SBUF state ----
            qk_sb = [pp.tile([128, T], f32r, name=f"qk{ct}") for ct in range(8)]
            # v plus a ones column per head: [t-part, head, t-tile, 65]
            v1_sb = pp.tile([128, HPC, NJ, D + 1], f32r, name="v1")
            bqk_sb = pp.tile([128, 8], f32, name="bqk")
            bv_sb = pp.tile([1, CC], f32r, name="bv")
            ones_sb = pp.tile([1, 128], f32r, name="ones")
            wqk_t = []
            wv_t = []

            # input DMAs: x(tb0) first so the first matmul group can start,
            # then weights, then the small vectors
            xs_tb = {}

            def load_x(tb):
                tsl = slice(tb * 512, (tb + 1) * 512)
                xs = []
                for e in range(ES):
                    xe = xp.tile([128, 512], f32r, tag=f"x{e}",
                                 bufs=(2 if e < 3 else 1), name=f"x{e}_{tb}")
                    nc.sync.dma_start(out=xe, in_=xT[e * 128 : (e + 1) * 128, tsl])
                    xs.append(xe)
                xs_tb[tb] = xs

            # small constants first, then x/w interleaved per e-tile so the
            # first matmul accumulation group can finish as early as possible
            nc.sync.dma_start(out=bqk_sb, in_=b_qk)
            nc.sync.dma_start(out=bv_sb, in_=b_v)
            nc.sync.dma_start(out=ones_sb, in_=ones_d)
            tsl0 = slice(0, 512)
            xs0 = []
            for e in range(ES):
                xe = xp.tile([128, 512], f32r, tag=f"x{e}",
                             bufs=(2 if e < 3 else 1), name=f"x{e}_0")
                nc.sync.dma_start(out=xe, in_=xT[e * 128 : (e + 1) * 128, tsl0])
                xs0.append(xe)
                wv = pp.tile([128, CC], f32r, name=f"wv{e}")
                nc.sync.dma_start(out=wv, in_=w_v[e * 128 : (e + 1) * 128, :])
                wv_t.append(wv)
            xs_tb[0] = xs0
            for e in range(ES):
                wqk = pp.tile([128, 2 * CC], f32r, name=f"wqk{e}")
                nc.sync.dma_start(out=wqk, in_=w_qk[e * 128 : (e + 1) * 128, :])
                wqk_t.append(wqk)
            ones_bc = _bcast_ap(ones_d, 128)
            nc.sync.dma_start(out=v1_sb[:, :, :, D : D + 1], in_=ones_bc)

            def qkv_group_qk(tb, ct):
                tsl = slice(tb * 512, (tb + 1) * 512)
                xs = xs_tb[tb]
                ps = psp.tile([128, 512], f32, tag="st", bufs=2,
                              name=f"psqk{ct}_{tb}")
                for e in range(ES):
                    nc.tensor.matmul(
                        ps,
                        wqk_t[e][:, ct * 128 : (ct + 1) * 128],
                        xs[e],
                        start=(e == 0),
                        stop=(e == ES - 1),
                    )
                nc.scalar.activation(
                    qk_sb[ct][:, tsl], ps, Act.Identity,
                    bias=bqk_sb[:, ct : ct + 1], scale=1.0)

            def qkv_group_v(tb, k4):
                xs = xs_tb[tb]
                tt = tb * 4 + k4
                psv = psp.tile([128, 512], f32, tag="st", bufs=2,
                               name=f"psv{tt}")
                nc.tensor.matmul(
                    psv, ones_sb, bv_sb,
                    start=True, stop=False, skip_group_check=True,
                )
                for e in range(ES):
                    nc.tensor.matmul(
                        psv,
                        xs[e][:, k4 * 128 : (k4 + 1) * 128],
                        wv_t[e],
                        start=False,
                        stop=(e == ES - 1),
                        skip_group_check=True,
                    )
                nc.vector.tensor_copy(
                    v1_sb[:, :, tt, 0:D],
                    psv.rearrange("p (h d) -> p h d", d=D),
                )

            def attn_block(I, nxt=()):
                isl = slice(I * 512, (I + 1) * 512)
                nj = 4 * I + 4  # causal j-tiles for this i-block
                yts = {}
                pts = {}

                def alloc_yt(pr):
                    yts[pr] = [
                        psp.tile([D + 1, 512], f32, tag=f"yt{n}",
                                 name=f"yt{n}_{pr}_{I}")
                        for n in ("A0", "A1", "B0", "B1")
                    ]

                def qk_exp(pr, J):
                    qt = qk_sb[pr]
                    kt = qk_sb[4 + pr]
                    jsl = slice(J * 128, (J + 1) * 128)
                    st = psp.tile([128, 1024], f32, tag="st", bufs=2,
                                  name=f"st{pr}_{I}_{J}")
                    # QK row-tile pair: head A rows 0-63, head B 64-127
                    nc.tensor.matmul(
                        st[:, 0:512], kt[0:64, jsl], qt[0:64, isl],
                        tile_position=(0, 0),
                    )
                    nc.tensor.matmul(
                        st[:, 512:1024], kt[64:128, jsl], qt[64:128, isl],
                        tile_position=(64, 0),
                    )
                    pt = ptp.tile([128, 1024], f32r, tag="pt",
                                  name=f"pt{pr}_{I}_{J}")
                    nc.scalar.activation(pt, st, Act.Exp, scale=0.125)
                    r = J - 4 * I
                    if r >= 0:  # diagonal tile: causal mask, trimmed width
                        w = (r + 1) * 128
                        for off in (0, 512):
                            # keep where (512I + y) - (128J + x) >= 0
                            nc.gpsimd.affine_select(
                                out=pt[:, off : off + w],
                                in_=pt[:, off : off + w],
                                compare_op=is_ge,
                                fill=0.0,
                                base=-128 * r,
                                pattern=[[1, w]],
                                channel_multiplier=-1,
                            )
                    pts[(pr, J)] = pt

                def pv(pr, J):
                    pt = pts.pop((pr, J))
                    ytA0, ytA1, ytB0, ytB1 = yts[pr]
                    first, last = (J == 0), (J == nj - 1)
                    # PV row-tile pairs (j contraction split 64+64)
                    nc.tensor.matmul(
                        ytA0, v1_sb[0:64, 2 * pr, J, :], pt[0:64, 0:512],
                        tile_position=(0, 0),
                        start=first, stop=last, skip_group_check=True,
                    )
                    nc.tensor.matmul(
                        ytA1, v1_sb[64:128, 2 * pr, J, :], pt[64:128, 0:512],
                        tile_position=(64, 0),
                        start=first, stop=last, skip_group_check=True,
                    )
                    nc.tensor.matmul(
                        ytB0, v1_sb[0:64, 2 * pr + 1, J, :], pt[0:64, 512:1024],
                        tile_position=(0, 0),
                        start=first, stop=last, skip_group_check=True,
                    )
                    nc.tensor.matmul(
                        ytB1, v1_sb[64:128, 2 * pr + 1, J, :],
                        pt[64:128, 512:1024],
                        tile_position=(64, 0),
                        start=first, stop=last, skip_group_check=True,
                    )

                def out_stage(pr):
                    ytA0, ytA1, ytB0, ytB1 = yts.pop(pr)
                    # ---- normalize + emit [128 rows = 2 heads, 512] ----
                    # copy + single add per head frees the yt psum slots after
                    # only two DVE ops; row 64 of tmp holds the softmax sum
                    ystage = op.tile([128, 512], f32, tag="ystage", bufs=2,
                                     name=f"ys{pr}_{I}")
                    rec2 = op.tile([33, 512], f32, tag="rec2",
                                   name=f"rec2{pr}_{I}")
                    tmpA = op.tile([D + 1, 512], f32, tag="tmpA",
                                   name=f"tmpA{pr}_{I}")
                    sum1 = op.tile([1, 512], f32, tag="sum1",
                                   name=f"sum1{pr}_{I}")
                    sA = op.tile([D + 1, 512], f32, tag="sA", name=f"sA{pr}_{I}")
                    sB = op.tile([D + 1, 512], f32, tag="sB", name=f"sB{pr}_{I}")
                    recA, recB = rec2[0:1, :], rec2[32:33, :]
                    # head A: combine into tmpA (base 0 throughout)
                    nc.vector.tensor_copy(sA, ytA1)
                    nc.vector.tensor_add(tmpA, ytA0, sA)
                    nc.vector.reciprocal(recA, tmpA[D : D + 1, :])
                    # head B: combine straight into ystage rows 64-127 so the
                    # final in-place mul keeps matching base partitions
                    nc.vector.tensor_copy(sB, ytB1)
                    nc.vector.tensor_add(ystage[64:128, :], ytB0[0:D, :],
                                         sB[0:D, :])
                    nc.vector.tensor_add(sum1, ytB0[D : D + 1, :],
                                         sB[D : D + 1, :])
                    nc.vector.reciprocal(recB, sum1)
                    # broadcast 1/sum across partitions via a DRAM bounce
                    # (keeps the PE stream free of output-stage work)
                    recA_d = dp.tile([1, 512], f32, tag="recA_d",
                                     name=f"recAd{pr}_{I}")
                    recB_d = dp.tile([1, 512], f32, tag="recB_d",
                                     name=f"recBd{pr}_{I}")
                    rbc2 = op.tile([128, 512], f32, tag="rbc2",
                                   name=f"rbc2{pr}_{I}")
                    rbcA, rbcB = rbc2[0:64, :], rbc2[64:128, :]
                    nc.sync.dma_start(out=recA_d, in_=recA)
                    nc.sync.dma_start(out=recB_d, in_=recB)
                    nc.sync.dma_start(out=rbcA, in_=_bcast_ap(recA_d, 64))
                    nc.sync.dma_start(out=rbcB, in_=_bcast_ap(recB_d, 64))
                    nc.vector.tensor_mul(ystage[0:64, :], tmpA[0:D, :], rbcA)
                    nc.vector.tensor_mul(ystage[64:128, :], ystage[64:128, :],
                                         rbcB)
                    nc.sync.dma_start(
                        out=yT[pr * 128 : (pr + 1) * 128, isl], in_=ystage)

                # 1-stage software pipeline across the whole block: QK(k+1)
                # issues before PV(k) so the PE never sits behind a PV that
                # is waiting on exp
                items = [(pr, J) for pr in range(PAIRS) for J in range(nj)]
                emitted = 0
                done = 0

                def emit_qk(k):
                    pr, J = items[k]
                    if J == 0:
                        alloc_yt(pr)
                    qk_exp(pr, J)

                # lookahead-1 pipeline, deepened to 2 at pair boundaries so
                # the first PV of a new pair isn't reached while the DVE is
                # still releasing the previous pair's yt slots
                nxt = list(nxt)
                stride = max(1, len(items) // len(nxt)) if nxt else 0
                emitted = 0
                # constant lookahead-2 pipeline: QK/exp for items k+1 and k+2
                # are already in the stream when PV(k) issues, so neither a
                # PV wait nor a woven QKV group ever starves ScalarE
                for k in range(len(items)):
                    while emitted < min(k + 4, len(items)):
                        emit_qk(emitted)
                        emitted += 1
                    pr, J = items[k]
                    pv(pr, J)
                    if J == nj - 1:
                        out_stage(pr)
                    # weave next t-block's QKV groups into the PE stream
                    if nxt and (k + 1) % stride == 0:
                        fn, a, b = nxt.pop(0)
                        fn(a, b)
                for fn, a, b in nxt:
                    fn(a, b)

            # schedule: per t-block, QKV projection then attention I = tb
            # (causal: block I only needs k/v from t-blocks <= I)
            for g in range(4):
                qkv_group_v(0, g)
            for g in range(8):
                qkv_group_qk(0, g)
            for I in range(TB):
                nxt = []
                if I + 1 < TB:
                    load_x(I + 1)
                    nxt = [(qkv_group_qk, I + 1, g) for g in range(8)] + [
                        (qkv_group_v, I + 1, g) for g in range(4)
                    ]
                attn_block(I, nxt)
    nc.compile()
    return nc


def _bcast_ap(src_ap, nparts):
    """Partition-broadcast view of a [1, N] DRAM AP -> [nparts, N]."""
    import concourse.bass as bass

    return bass.AP(
        tensor=src_ap.tensor,
        offset=src_ap.offset,
        ap=[[0, nparts]] + list(src_ap.ap)[1:],
    )


def get_nc():
    if "nc" not in _cache:
        _cache["nc"] = _build_nc()
    return _cache["nc"]


def shard_inputs(x, w_attn, b_attn):
    """Full inputs -> per-core input maps (host-side slicing/transposition)."""
    x = np.asarray(x, dtype=np.float32)
    w = np.asarray(w_attn, dtype=np.float32)
    bb = np.asarray(b_attn, dtype=np.float32)
    in_maps = []
    for core in range(N_CORES):
        b, hg = core // 2, core % 2
        r0 = hg * CC  # first q row for this head group
        w_qk = np.ascontiguousarray(
            np.concatenate([w[r0 : r0 + CC, :], w[C + r0 : C + r0 + CC, :]], axis=0).T
        )
        w_v = np.ascontiguousarray(w[2 * C + r0 : 2 * C + r0 + CC, :].T)
        b_qk = np.stack(
            [bb[r0 + ct * 128 : r0 + (ct + 1) * 128] for ct in range(4)]
            + [bb[C + r0 + ct * 128 : C + r0 + (ct + 1) * 128] for ct in range(4)],
            axis=1,
        ).astype(np.float32)
        b_v = bb[2 * C + r0 : 2 * C + r0 + CC].reshape(1, CC).astype(np.float32)
        in_maps.append(
            {
                "xT": np.ascontiguousarray(x[b].T),
                "w_qk": w_qk,
                "w_v": w_v,
                "b_qk": np.ascontiguousarray(b_qk),
                "b_v": np.ascontiguousarray(b_v),
                "ones_d": np.ones((1, 128), dtype=np.float32),
            }
        )
    return in_maps


def run(in_maps, trace=False, **kw):
    from concourse import bass_utils

    nc = get_nc()
    return bass_utils.run_bass_kernel_spmd(
        nc, in_maps, core_ids=list(range(N_CORES)), trace=trace, **kw
    )


def gather_output(results):
    y = np.empty((B, T, E), dtype=np.float32)
    for core in range(N_CORES):
        b, hg = core // 2, core % 2
        y[b, :, hg * CC : (hg + 1) * CC] = results[core]["yT"].T
    return y


def kernel(x, w_attn, b_attn):
    in_maps = shard_inputs(x, w_attn, b_attn)
    res = run(in_maps, trace=False)
    return gather_output(res.results)
